# revision 24
# baseline (speedup 1.0000x reference)
"""Multi-head attention (B=2, S=2048, D=2048, H=16) on 8 TRN2 NeuronCores.

Sharding: data-parallel over batch (2) x Megatron tensor-parallel over heads
(4 groups of 4 heads). Core c = 4*b + g handles batch b, heads [4g, 4g+4).
Each core computes q/k/v projections for its head slice, attention over its
4 heads, and a partial o_proj contribution; the host sums the 4 partials per
batch (the unshard step of Megatron TP) and stacks the 2 batches.

All matmuls are fp32r (TF32-like, full PE rate) with 512-wide moving
operands. k, v and q all stay resident in SBUF between projection and
attention -- no DRAM spill. SBUF stays within budget via phase-scoped
pools; the x pool is shared by B1 and B2 so B2's x prefetch overlaps B1.

Pipelining for the in-order engine queues:
 - B1 orders v before k (and wv below wk in the pool) so wq's DMA can land
   in wv's freed range while B1's k groups still run.
 - x and weight DMAs dispatch from different engines (sync vs scalar).
 - In C, attn@v is skewed one kt tile behind scores; the softmax tail of
   head h (denominator colsum on PE, broadcast on GPSIMD,
   reciprocal on DVE, normalize on GPSIMD) is injected at kt=6/8/10/12 of
   head h+1's loop; o_proj of chunk qc runs between the heads of qc+1.
 - Denominator accumulation: DVE chain (11 ops) + GPSIMD chain (5 ops),
   matching their throughput ratio.
"""

import math
import os

import numpy as np
import ml_dtypes

import concourse.mybir as mybir
import concourse.tile as tile
from concourse import bacc
from concourse.bass_utils import run_bass_kernel_spmd

F32 = mybir.dt.float32
F32R = mybir.dt.float32r
BF16 = mybir.dt.bfloat16

B, S, D = 2, 2048, 2048
H = 16
HD = 128
G = 4            # tensor-parallel groups
HLOC = H // G    # heads per core = 4
DG = HLOC * HD   # per-core projection width = 512
P = 128
NCORES = 8

SCHUNK = 512
NSC = S // SCHUNK          # 4 chunks
DT = D // P                # 16 contraction tiles
MT = DG // P               # 4 output tiles (= heads) per projection
KT = S // P                # 16 key tiles
QC = NSC                   # 4 query chunks
IC = D // SCHUNK           # 4 o_proj output column blocks
ST_PER_CHUNK = SCHUNK // P # 4 row tiles per chunk
INV_SQRT_HD = 1.0 / math.sqrt(HD)
GPS_C = (1, 4, 7, 10)            # GPSIMD denominator chain
DVE_A = (0, 2, 3, 5, 12, 14)     # DVE denominator chain A
DVE_B = (6, 8, 9, 11, 13, 15)    # DVE denominator chain B

_cache = {}
last_run = None  # BassKernelResults of the most recent execution (for test.py)


def build(loop_reps=None):
    nc = bacc.Bacc(None, target_bir_lowering=False)

    xT_dr = nc.dram_tensor("xT", [D, S], F32R, kind="ExternalInput")
    wkT_dr = nc.dram_tensor("wkT", [D, DG], F32R, kind="ExternalInput")
    wqT_dr = nc.dram_tensor("wqT", [D, DG], F32R, kind="ExternalInput")
    wvT_dr = nc.dram_tensor("wvT", [D, DG], F32R, kind="ExternalInput")
    woT_dr = nc.dram_tensor("woT", [DG, D], BF16, kind="ExternalInput")
    out_d = nc.dram_tensor("out", [S, D], F32, kind="ExternalOutput")

    xT_view = xT_dr.rearrange("(o p) s -> p o s", p=P)
    wkT_v = wkT_dr.rearrange("(o p) m -> p o m", p=P)
    wqT_v = wqT_dr.rearrange("(o p) m -> p o m", p=P)
    wvT_v = wvT_dr.rearrange("(o p) m -> p o m", p=P)
    woT_v = woT_dr.rearrange("(o p) i -> p o i", p=P)

    import contextlib

    with tile.TileContext(nc) as tc:
        loop_cm = tc.For_i(0, loop_reps, 1) if loop_reps else contextlib.nullcontext()
        with loop_cm:
            with (
                tc.tile_pool(name="ktres", bufs=1) as ktpool,
                tc.tile_pool(name="vres", bufs=1) as vpool,
            ):
                # k^T resident: [HD part, head, key]
                kT = ktpool.tile([P, HLOC, S], BF16, tag="kT")
                # v resident: [key-in-tile part, key tile, head, hd]
                # bf16: the attn@v matmul runs bf16 (same PE rate at N=512),
                # and the exp/denominator path gets 2x DVE throughput
                vv = vpool.tile([P, KT, HLOC, HD], BF16, tag="vv")

                # ---------- B1: v and k projections ----------
                with (
                    tc.tile_pool(name="wkv", bufs=1) as wpool,
                    tc.tile_pool(name="xt1", bufs=2) as xtpool,
                    tc.tile_pool(name="psumB1", bufs=1, space="PSUM") as psumB,
                ):
                    # first x chunk, finest-grained first piece, on sync
                    xt0 = xtpool.tile([P, DT, SCHUNK], F32R, tag="xt")
                    nc.sync.dma_start(xt0[:, 0:1], xT_view[:, 0:1, 0:SCHUNK])
                    nc.sync.dma_start(xt0[:, 1:4], xT_view[:, 1:4, 0:SCHUNK])
                    for d0 in range(4, DT, 4):
                        nc.sync.dma_start(
                            xt0[:, d0:d0 + 4], xT_view[:, d0:d0 + 4, 0:SCHUNK])

                    # wv FIRST (lower address) so wq can reuse its range while
                    # B1's k groups still read wk; weight DMAs on scalar
                    wvT = wpool.tile([P, DT, DG], F32R, tag="wvT")
                    wkT = wpool.tile([P, DT, DG], F32R, tag="wkT")
                    nc.scalar.dma_start(wvT[:, 0:1], wvT_v[:, 0:1])
                    nc.scalar.dma_start(wvT[:, 1:4], wvT_v[:, 1:4])
                    for d0 in range(4, DT, 4):
                        nc.scalar.dma_start(wvT[:, d0:d0 + 4], wvT_v[:, d0:d0 + 4])
                    for d0 in range(0, DT, 4):
                        nc.scalar.dma_start(wkT[:, d0:d0 + 4], wkT_v[:, d0:d0 + 4])

                    for sc in range(NSC):
                        c0 = sc * SCHUNK
                        if sc == 0:
                            xt = xt0
                        else:
                            xt = xtpool.tile([P, DT, SCHUNK], F32R, tag="xt")
                            for d0 in range(0, DT, 4):
                                nc.sync.dma_start(
                                    xt[:, d0:d0 + 4],
                                    xT_view[:, d0:d0 + 4, c0:c0 + SCHUNK])

                        def v_groups():
                            # v projection: out tile [keys 128, (head, hd) 512]
                            for st in range(ST_PER_CHUNK):
                                ps = psumB.tile([P, DG], F32, tag="psB", bufs=4,
                                                name="psv")
                                for dt in range(DT):
                                    nc.tensor.matmul(
                                        ps[:], xt[:, dt, st * P:(st + 1) * P],
                                        wvT[:, dt, :],
                                        start=(dt == 0), stop=(dt == DT - 1))
                                kt_idx = sc * ST_PER_CHUNK + st
                                nc.vector.tensor_copy(
                                    vv[:, kt_idx].rearrange("p h n -> p (h n)"),
                                    ps[:])

                        def k_groups():
                            # k projection: out tile [head-dims 128, 512 keys]
                            for mt in range(MT):
                                ps = psumB.tile([P, SCHUNK], F32, tag="psB",
                                                bufs=4, name="psk")
                                for dt in range(DT):
                                    nc.tensor.matmul(
                                        ps[:], wkT[:, dt, mt * P:(mt + 1) * P],
                                        xt[:, dt, :],
                                        start=(dt == 0), stop=(dt == DT - 1))
                                nc.vector.tensor_copy(
                                    kT[:, mt, c0:c0 + SCHUNK], ps[:])

                        # last chunk runs k first so wk's range frees early
                        # for wq to land in during B1's tail
                        if sc == NSC - 1:
                            k_groups(); v_groups()
                        else:
                            v_groups(); k_groups()

                with tc.tile_pool(name="qtres", bufs=1) as qtpool:
                    # q^T resident: [HD part, head, query]
                    qT = qtpool.tile([P, HLOC, S], BF16, tag="qT")

                    # ---------- B2: q projection ----------
                    with (
                        tc.tile_pool(name="wq", bufs=1) as wqpool,
                        tc.tile_pool(name="xt2", bufs=2) as xtpool2,
                        tc.tile_pool(name="psumB2", bufs=1, space="PSUM") as psumB2,
                    ):
                        wqT = wqpool.tile([P, DT, DG], F32R, tag="wqT")
                        for d0 in range(0, DT, 4):
                            nc.scalar.dma_start(wqT[:, d0:d0 + 4], wqT_v[:, d0:d0 + 4])

                        for sc in range(NSC):
                            c0 = sc * SCHUNK
                            xt = xtpool2.tile([P, DT, SCHUNK], F32R, tag="xt2")
                            for d0 in range(0, DT, 4):
                                nc.sync.dma_start(
                                    xt[:, d0:d0 + 4],
                                    xT_view[:, d0:d0 + 4, c0:c0 + SCHUNK])
                            for mt in range(MT):
                                ps = psumB2.tile([P, SCHUNK], F32, tag="psB2",
                                                 bufs=4)
                                for dt in range(DT):
                                    nc.tensor.matmul(
                                        ps[:], wqT[:, dt, mt * P:(mt + 1) * P],
                                        xt[:, dt, :],
                                        start=(dt == 0), stop=(dt == DT - 1))
                                nc.vector.tensor_copy(
                                    qT[:, mt, c0:c0 + SCHUNK], ps[:])

                    # ---------- C: attention + o_proj ----------
                    with (
                        tc.tile_pool(name="wo2", bufs=1) as wopool,
                        tc.tile_pool(name="ctx", bufs=2) as ctxpool,
                        tc.tile_pool(name="expp", bufs=12) as expool,
                        tc.tile_pool(name="accp", bufs=2) as accpool,
                        tc.tile_pool(name="asmall", bufs=2) as small,
                        tc.tile_pool(name="ostg", bufs=3) as ostg,
                        tc.tile_pool(name="pss", bufs=1, space="PSUM") as psums,
                        tc.tile_pool(name="pso", bufs=1, space="PSUM") as psumo,
                        tc.tile_pool(name="psd", bufs=1, space="PSUM") as psumd,
                        tc.tile_pool(name="psj", bufs=1, space="PSUM") as psumj,
                    ):
                        woT = wopool.tile([P, MT, D], BF16, tag="woT")
                        for j0 in range(MT):
                            nc.sync.dma_start(woT[:, j0:j0 + 1], woT_v[:, j0:j0 + 1])

                        ones_b = small.tile([P, 1], BF16, tag="ones_b", bufs=1)
                        nc.vector.memset(ones_b[:], 1.0)

                        # softmax tails in flight (up to 2 heads deep)
                        from collections import deque
                        tails = deque()
                        # o_proj groups ready to emit
                        oproj_queue = deque()

                        def tail_step(t, step):
                            if step == 0:
                                # denominator colsum on PE (fp32 psum, exact)
                                t["pssum"] = psumd.tile(
                                    [1, SCHUNK], F32, tag="pssum", bufs=1,
                                    name="pssum_t")
                                nc.tensor.matmul(
                                    t["pssum"][:], ones_b[:], t["accA"][:],
                                    start=True, stop=False)
                                nc.tensor.matmul(
                                    t["pssum"][:], ones_b[:], t["accB"][:],
                                    start=False, stop=False)
                                nc.tensor.matmul(
                                    t["pssum"][:], ones_b[:], t["acc2"][:],
                                    start=False, stop=True)
                            elif step == 1:
                                # stage sums to SBUF on ACT (GPSIMD can't
                                # read PSUM)
                                t["sden"] = small.tile([1, SCHUNK], F32,
                                                       tag="sden", name="sden_t")
                                nc.scalar.copy(t["sden"][:], t["pssum"][:])
                            elif step == 2:
                                # ~5x faster than reciprocal(); ~18 bits is
                                # ample for softmax denominators
                                t["rcp"] = small.tile([1, SCHUNK], F32,
                                                      tag="rcp", name="rcp_t")
                                nc.vector.reciprocal_approx_fast(
                                    out=t["rcp"][:], in_=t["sden"][:])
                            elif step == 3:
                                t["rb"] = small.tile([P, SCHUNK], F32, tag="rb",
                                                     name="rb_t")
                                nc.gpsimd.partition_broadcast(
                                    t["rb"][:], t["rcp"][:])
                            elif step == 4:
                                nc.vector.tensor_mul(
                                    t["ctx_slice"], t["pso"][:], t["rb"][:])

                        def oproj_group(octx, oqc, st):
                            stile = oqc * ST_PER_CHUNK + st
                            for ic in range(IC):
                                ps = psumj.tile([P, SCHUNK], F32,
                                                tag="opsum", bufs=2)
                                for jt in range(MT):
                                    nc.tensor.matmul(
                                        ps[:],
                                        octx[:, jt, st * P:(st + 1) * P],
                                        woT[:, jt,
                                            ic * SCHUNK:(ic + 1) * SCHUNK],
                                        start=(jt == 0), stop=(jt == MT - 1))
                                ob = ostg.tile([P, SCHUNK], F32, tag="ostage")
                                if ic == 3:
                                    nc.scalar.copy(ob[:], ps[:])
                                else:
                                    nc.vector.tensor_copy(ob[:], ps[:])
                                nc.sync.dma_start(
                                    out_d[stile * P:(stile + 1) * P,
                                          ic * SCHUNK:(ic + 1) * SCHUNK],
                                    ob[:])

                        lp = nc.allow_low_precision(
                            reason="bf16 denominator chains: depth<=6, "
                                   "rounding ~0.3% vs 2e-2 gate")
                        lp.__enter__()
                        for qc in range(QC):
                            q0 = qc * SCHUNK
                            ctx = ctxpool.tile([P, HLOC, SCHUNK], BF16, tag="ctx")
                            for h in range(HLOC):
                                accA = accpool.tile([P, SCHUNK], BF16, tag="accA")
                                accB = accpool.tile([P, SCHUNK], BF16, tag="accB")
                                acc2 = accpool.tile([P, SCHUNK], BF16, tag="acc2")
                                pso = psumo.tile([P, SCHUNK], F32, tag="pso",
                                                 bufs=3)
                                prev_exp = None
                                for kt in range(KT):
                                    pss = psums.tile([P, SCHUNK], F32, tag="pss",
                                                     bufs=2)
                                    nc.tensor.matmul(
                                        pss[:], kT[:, h, kt * P:(kt + 1) * P],
                                        qT[:, h, q0:q0 + SCHUNK],
                                        start=True, stop=True)
                                    # attn@v skewed one tile behind scores so
                                    # the in-order PE never waits on ACT's exp
                                    if kt >= 1:
                                        nc.tensor.matmul(
                                            pso[:], vv[:, kt - 1, h, :],
                                            prev_exp[:],
                                            start=(kt == 1), stop=False)
                                    expP = expool.tile([P, SCHUNK], BF16,
                                                       tag="expP")
                                    nc.scalar.activation(
                                        expP[:], pss[:],
                                        mybir.ActivationFunctionType.Exp,
                                        scale=INV_SQRT_HD)
                                    # 3 denominator chains (bf16): 2 on DVE,
                                    # 1 on GPSIMD; capped depth bounds the
                                    # bf16 partial-sum rounding
                                    if kt == DVE_A[0]:
                                        nc.vector.tensor_copy(accA[:], expP[:])
                                    elif kt == DVE_B[0]:
                                        nc.vector.tensor_copy(accB[:], expP[:])
                                    elif kt == GPS_C[0]:
                                        nc.gpsimd.tensor_copy(acc2[:], expP[:])
                                    elif kt in GPS_C:
                                        nc.gpsimd.tensor_add(acc2[:], acc2[:], expP[:])
                                    elif kt in DVE_A:
                                        nc.vector.tensor_add(accA[:], accA[:], expP[:])
                                    else:
                                        nc.vector.tensor_add(accB[:], accB[:], expP[:])
                                    prev_exp = expP
                                    # tail pipeline, two heads deep: head h-1
                                    # runs steps 0/1 (colsum, stage) and 2/3
                                    # (recip late, broadcast) here; head h-2
                                    # finishes its normalize (step 4) early
                                    if len(tails) >= 2 and kt == 2:
                                        t0_ = tails.popleft()
                                        tail_step(t0_, 4)
                                        if t0_["last_head"]:
                                            for st_ in range(ST_PER_CHUNK):
                                                oproj_queue.append(
                                                    (t0_["ctx"], t0_["oqc"], st_))
                                    if tails and tails[-1]["steps"] == 0 and kt == 6:
                                        tail_step(tails[-1], 0)
                                        tails[-1]["steps"] = 1
                                    elif tails and tails[-1]["steps"] == 1 and kt == 8:
                                        tail_step(tails[-1], 1)
                                        tails[-1]["steps"] = 2
                                    elif tails and tails[-1]["steps"] == 2 and kt == 14:
                                        tail_step(tails[-1], 2)
                                        tails[-1]["steps"] = 3
                                    elif tails and tails[-1]["steps"] == 3 and kt == 15:
                                        tail_step(tails[-1], 3)
                                        tails[-1]["steps"] = 4
                                nc.tensor.matmul(
                                    pso[:], vv[:, KT - 1, h, :], prev_exp[:],
                                    start=False, stop=True)
                                tails.append({
                                    "accA": accA, "accB": accB, "acc2": acc2,
                                    "pso": pso, "ctx_slice": ctx[:, h, :],
                                    "steps": 0, "ctx": ctx, "oqc": qc,
                                    "last_head": h == HLOC - 1,
                                })
                                # o_proj groups whose ctx is fully
                                # normalized (emission-order safe)
                                if oproj_queue:
                                    octx_, oqc_, st_ = oproj_queue.popleft()
                                    oproj_group(octx_, oqc_, st_)

                        # drain remaining tails, then o_proj
                        while tails:
                            t = tails.popleft()
                            for step in range(t["steps"], 5):
                                tail_step(t, step)
                            if t["last_head"]:
                                for st_ in range(ST_PER_CHUNK):
                                    oproj_queue.append(
                                        (t["ctx"], t["oqc"], st_))
                        while oproj_queue:
                            octx_, oqc_, st_ = oproj_queue.popleft()
                            oproj_group(octx_, oqc_, st_)
                        lp.__exit__(None, None, None)

    nc.finalize()
    return nc

_build = build


def _round_f32r(a):
    """Round fp32 to fp32r bit patterns (round-to-nearest-even to 12 explicit
    mantissa bits, TF32-like) -- matches the hardware's own rounding."""
    u = np.ascontiguousarray(a, dtype=np.float32).view(np.uint32)
    keep = np.uint32(0xFFFFF000)
    half = np.uint32(0x7FF)
    lsb = (u >> np.uint32(12)) & np.uint32(1)
    return ((u + half + lsb) & keep).view(np.float32)


def kernel(hidden_states, wq, wk, wv, wo):
    global last_run
    if "nc" not in _cache:
        _cache["nc"] = build()
    nc = _cache["nc"]

    hidden_states = np.asarray(hidden_states, dtype=np.float32)
    wq = np.asarray(wq, dtype=np.float32)
    wk = np.asarray(wk, dtype=np.float32)
    wv = np.asarray(wv, dtype=np.float32)
    wo = np.asarray(wo, dtype=np.float32)

    xT = [_round_f32r(hidden_states[b].T) for b in range(B)]
    in_maps = []
    for c in range(NCORES):
        b, g = divmod(c, G)
        sl = slice(g * DG, (g + 1) * DG)
        in_maps.append({
            "xT": xT[b],
            "wqT": _round_f32r(wq[sl, :].T),
            "wkT": _round_f32r(wk[sl, :].T),
            "wvT": _round_f32r(wv[sl, :].T),
            "woT": np.ascontiguousarray(wo[:, sl].T).astype(ml_dtypes.bfloat16),
        })

    trace = os.environ.get("BASSKERNEL_TRACE", "0") == "1"
    last_run = run_bass_kernel_spmd(
        nc, in_maps, core_ids=list(range(NCORES)), trace=trace)

    out = np.empty((B, S, D), dtype=np.float32)
    for b in range(B):
        acc = None
        for g in range(G):
            part = last_run.results[b * G + g]["out"]
            acc = part.copy() if acc is None else acc + part
        out[b] = acc
    return out
